# revision 2
# baseline (speedup 1.0000x reference)
"""Trainium2 Bass kernel for nn_LC1GNS (LC-circuit graph network + Hamiltonian
gradient Euler step), data-parallel over 8 NeuronCores.

The per-element network is fused host-side into 10 small affine stages
(block matrices over a 113-row state: 7 chains x 16 latents + a ones row that
carries all biases through the relu matmuls). The batch streams along the
free dimension, 512 elements per tile.
"""
import os, sys, json

sys.path.insert(0, "/opt/trn_rl_repo")
import numpy as np

import concourse.bass as bass
import concourse.tile as tile
import concourse.mybir as mybir
from concourse.bass_utils import run_bass_kernel_spmd

L = 16
DT_EULER = 0.01
B_FULL = 262144
N_CORES = 8
S = B_FULL // N_CORES  # 32768 per core
TILE_N = 512
N_TILES = S // TILE_N

# matmul input dtype for stages 2..10 ("f32r" = 1 cyc/row, "f32" = 4 cyc/row)
MM_DTYPE = os.environ.get("LC_MM_DTYPE", "f32r")

ROW_Q, ROW_PHI, ROW_CE, ROW_N0, ROW_N1, ROW_N2, ROW_CTRL, ROW_ONE = range(8)


def build_mats(params):
    """Fold the reference GNN into lhsT ([K, M]) stage matrices."""
    def A(x):
        return np.asarray(x, dtype=np.float64)

    encp = [params["enc_C"], params["enc_L"], params["enc_C"], params["enc_I"],
            params["enc_node"], params["enc_node"], params["enc_node"]]
    decp = [params["dec_C"], params["dec_L"], params["dec_C"], params["dec_I"],
            params["dec_node"], params["dec_node"], params["dec_node"]]
    procp = params["proc_node"]

    xrow = [ROW_Q, ROW_PHI, ROW_Q, ROW_CE, ROW_N0, ROW_N1, ROW_N2]
    sgn = [1.0, 1.0, -1.0, 1.0, 1.0, 1.0, 1.0]

    W1t = np.zeros((8, 113))
    for c in range(7):
        W1, b1 = A(encp[c][0][0]), A(encp[c][0][1])
        W1t[xrow[c], 16 * c:16 * c + 16] += sgn[c] * W1[0]
        W1t[ROW_ONE, 16 * c:16 * c + 16] += b1
    W1t[ROW_ONE, 112] = 1.0

    E = [A(encp[c][1][0]) for c in range(7)]   # enc layer-2 weights [16,16]
    eb = [A(encp[c][1][1]) for c in range(7)]  # enc layer-2 biases

    P1, pb1 = A(procp[0][0]), A(procp[0][1])   # [48,16], [16]
    PL2, pb2 = A(procp[1][0]), A(procp[1][1])  # [16,16], [16]
    RECV, SEND = [1, 2, 2, 2], [0, 1, 0, 2]

    W2t = np.zeros((113, 49))
    for i in range(3):
        cols = slice(16 * i, 16 * i + 16)
        # own node latent -> cat[0:16]
        W2t[16 * (4 + i):16 * (4 + i) + 16, cols] += E[4 + i] @ P1[0:16]
        W2t[112, cols] += eb[4 + i] @ P1[0:16]
        for e in range(4):
            if RECV[e] == i:
                W2t[16 * e:16 * e + 16, cols] += E[e] @ P1[16:32]
                W2t[112, cols] += eb[e] @ P1[16:32]
            if SEND[e] == i:
                W2t[16 * e:16 * e + 16, cols] += E[e] @ P1[32:48]
                W2t[112, cols] += eb[e] @ P1[32:48]
        W2t[112, cols] += pb1
    W2t[112, 48] = 1.0

    W3at = np.zeros((113, 113))
    for c in range(4):
        V1, c1 = A(decp[c][0][0]), A(decp[c][0][1])
        W3at[16 * c:16 * c + 16, 16 * c:16 * c + 16] = E[c] @ V1
        W3at[112, 16 * c:16 * c + 16] = eb[c] @ V1 + c1
    W3at[112, 112] = 1.0

    W3bt = np.zeros((49, 113))
    for i in range(3):
        V1, c1 = A(decp[4 + i][0][0]), A(decp[4 + i][0][1])
        W3bt[16 * i:16 * i + 16, 64 + 16 * i:64 + 16 * i + 16] = PL2 @ V1
        W3bt[48, 64 + 16 * i:64 + 16 * i + 16] = pb2 @ V1 + c1

    W4t = np.zeros((113, 113))
    for c in range(7):
        V2, c2 = A(decp[c][1][0]), A(decp[c][1][1])
        W4t[16 * c:16 * c + 16, 16 * c:16 * c + 16] = V2
        W4t[112, 16 * c:16 * c + 16] = c2
    W4t[112, 112] = 1.0

    WB1t = np.zeros((48, 48))
    WB2t = np.zeros((48, 48))
    for c in range(3):
        V1 = A(decp[c][0][0])
        V2 = A(decp[c][1][0])
        v3 = A(decp[c][2][0])[:, 0]
        # g_d1[u] = sum_j V2[u,j] * v3[j] * S3[j]
        WB1t[16 * c:16 * c + 16, 16 * c:16 * c + 16] = (V2 * v3[None, :]).T
        # gh1[t] = sum_u (E V1)[t,u] * B2[u]
        WB2t[16 * c:16 * c + 16, 16 * c:16 * c + 16] = (E[c] @ V1).T

    P1t = np.zeros((113, 8))
    for i in range(3):
        v3, c3 = A(decp[4 + i][2][0])[:, 0], A(decp[4 + i][2][1])
        P1t[16 * (4 + i):16 * (4 + i) + 16, i] = v3
        P1t[112, i] += c3[0]
    for c in range(3):
        v3, c3 = A(decp[c][2][0])[:, 0], A(decp[c][2][1])
        P1t[16 * c:16 * c + 16, 7] = v3
        P1t[112, 7] += c3[0]

    P2t = np.zeros((48, 8))
    W1C = A(encp[0][0][0])[0]
    W1L = A(encp[1][0][0])[0]
    P2t[16:32, 3] = DT_EULER * W1L       # ns0 += DT*dH/dphi
    P2t[0:16, 4] = -DT_EULER * W1C       # ns1 -= DT*gx0
    P2t[32:48, 4] = DT_EULER * W1C       # ns1 += DT*gx2
    P2t[16:32, 5] = -DT_EULER * W1L      # ns2 = -ns0

    P3t = np.zeros((8, 8))
    P3t[ROW_Q, 3] = 1.0
    P3t[ROW_CE, 3] = DT_EULER
    P3t[ROW_PHI, 4] = 1.0
    P3t[ROW_Q, 5] = -1.0
    P3t[ROW_CE, 5] = -DT_EULER
    P3t[ROW_CTRL, 6] = 1.0

    mats = dict(W1t=W1t, W2t=W2t, W3at=W3at, W3bt=W3bt, W4t=W4t,
                WB1t=WB1t, WB2t=WB2t, P1t=P1t, P2t=P2t, P3t=P3t)
    return {k: v.astype(np.float32) for k, v in mats.items()}


def host_pipeline(mats, IN):
    """Numpy model of the exact on-chip pipeline (for validation)."""
    relu = lambda x: np.maximum(x, 0.0)
    A0 = relu(mats["W1t"].T @ IN)
    A1 = relu(mats["W2t"].T @ A0)
    A2 = relu(mats["W3at"].T @ A0 + mats["W3bt"].T @ A1)
    A3 = relu(mats["W4t"].T @ A2)
    S3 = (A3[0:48] > 0).astype(np.float32)
    B2 = (A2[0:48] > 0) * (mats["WB1t"].T @ S3)
    B3 = (A0[0:48] > 0) * (mats["WB2t"].T @ B2)
    return mats["P1t"].T @ A3 + mats["P2t"].T @ B3 + mats["P3t"].T @ IN


def make_in_all(nodes, edges, control):
    """[8, n] host-packed input plane: q, phi, ce, n0..n2, control, ones."""
    n = control.shape[0]
    IN = np.empty((8, n), dtype=np.float32)
    IN[ROW_Q] = edges[:, 0, 0]
    IN[ROW_PHI] = edges[:, 1, 0]
    IN[ROW_CE] = edges[:, 3, 0]
    IN[ROW_N0] = nodes[:, 0, 0]
    IN[ROW_N1] = nodes[:, 1, 0]
    IN[ROW_N2] = nodes[:, 2, 0]
    IN[ROW_CTRL] = control
    IN[ROW_ONE] = 1.0
    return IN


# --- BIR post-pass: this container's walrus rejects >1 sync-wait per
# instruction; move excess waits onto injected NoOps earlier in the stream.
def _split_waits(bj_bytes):
    bj = json.loads(bj_bytes)
    engines = {"PE", "DVE", "Activation", "Pool", "SP"}
    tmpl_by_engine = {}
    for fn in bj["functions"]:
        for blk in fn["blocks"]:
            for inst in blk.get("instructions", []):
                if inst.get("opcode") == "NoOp" and inst.get("engine") in engines:
                    tmpl_by_engine.setdefault(inst["engine"], inst)
    ctr = [0]
    for fn in bj["functions"]:
        for blk in fn["blocks"]:
            insts = blk.get("instructions", [])
            out = []
            for inst in insts:
                si = inst.get("sync_info") or {}
                waits = si.get("on_wait") or []
                eng = inst.get("engine")
                limit = 0 if inst.get("opcode") == "Matmult" else 1
                if eng in engines and len(waits) > limit and eng in tmpl_by_engine:
                    keep, extra = waits[:limit], waits[limit:]
                    for w in extra:
                        nop = json.loads(json.dumps(tmpl_by_engine[eng]))
                        ctr[0] += 1
                        nop["name"] = f"I-waitfix-{ctr[0]}"
                        nop["sync_info"] = {"on_wait": [w], "on_update": []}
                        out.append(nop)
                    si["on_wait"] = keep
                    inst["sync_info"] = si
                out.append(inst)
            blk["instructions"] = out
    return json.dumps(bj).encode()


def build_nc():
    f32 = mybir.dt.float32
    mmdt = mybir.dt.float32r if MM_DTYPE == "f32r" else f32
    nc = bass.Bass()

    din = nc.dram_tensor("inall", [8, S], f32, kind="ExternalInput")
    wnames = dict(W1t=(8, 113), W2t=(113, 49), W3at=(113, 113), W3bt=(49, 113),
                  W4t=(113, 113), WB1t=(48, 48), WB2t=(48, 48),
                  P1t=(113, 8), P2t=(48, 8), P3t=(8, 8))
    dws = {k: nc.dram_tensor(k, list(sh), f32, kind="ExternalInput")
           for k, sh in wnames.items()}
    dy = nc.dram_tensor("y8", [8, S], f32, kind="ExternalOutput")

    with tile.TileContext(nc) as tc:
        with (
            tc.tile_pool(name="consts", bufs=1) as cp,
            tc.tile_pool(name="work", bufs=3) as wp,
            tc.tile_pool(name="ps", bufs=1, space="PSUM") as ps,
        ):
            # seed NoOp templates for the wait-split post-pass
            nc.vector.nop(); nc.scalar.nop(); nc.tensor.nop()
            nc.gpsimd.nop(); nc.sync.nop()

            wsb = {}
            for k, sh in wnames.items():
                t32 = cp.tile(list(sh), f32, tag=f"{k}_raw")
                nc.sync.dma_start(t32[:], dws[k][:])
                if mmdt is not f32:
                    tr = cp.tile(list(sh), mmdt, tag=f"{k}_r")
                    nc.vector.tensor_copy(tr[:], t32[:])
                    wsb[k] = tr
                else:
                    wsb[k] = t32

            for i in range(N_TILES):
                sl = bass.ts(i, TILE_N)
                tin = wp.tile([8, TILE_N], f32, tag="tin")
                nc.sync.dma_start(tin[:], din[:, sl])
                if mmdt is not f32:
                    tinr = wp.tile([8, TILE_N], mmdt, tag="tinr")
                    nc.vector.tensor_copy(tinr[:], tin[:])
                else:
                    tinr = tin

                Relu = mybir.ActivationFunctionType.Relu

                p0 = ps.tile([113, TILE_N], f32, tag="p0")
                nc.tensor.matmul(p0[:], wsb["W1t"][:], tinr[:], start=True, stop=True)
                A0 = wp.tile([113, TILE_N], mmdt, tag="A0")
                nc.scalar.activation(A0[:], p0[:], Relu)

                p1 = ps.tile([49, TILE_N], f32, tag="p1")
                nc.tensor.matmul(p1[:], wsb["W2t"][:], A0[:], start=True, stop=True)
                A1 = wp.tile([49, TILE_N], mmdt, tag="A1")
                nc.scalar.activation(A1[:], p1[:], Relu)

                p2 = ps.tile([113, TILE_N], f32, tag="p2")
                nc.tensor.matmul(p2[:], wsb["W3at"][:], A0[:], start=True, stop=False)
                nc.tensor.matmul(p2[:], wsb["W3bt"][:], A1[:], start=False, stop=True)
                A2 = wp.tile([113, TILE_N], mmdt, tag="A2")
                nc.scalar.activation(A2[:], p2[:], Relu)

                p3 = ps.tile([113, TILE_N], f32, tag="p3")
                nc.tensor.matmul(p3[:], wsb["W4t"][:], A2[:], start=True, stop=True)
                A3 = wp.tile([113, TILE_N], mmdt, tag="A3")
                nc.scalar.activation(A3[:], p3[:], Relu)

                S3 = wp.tile([48, TILE_N], mmdt, tag="S3")
                nc.vector.tensor_scalar(S3[:], A3[0:48, :], 0.0, None,
                                        mybir.AluOpType.is_gt)

                p4 = ps.tile([48, TILE_N], f32, tag="p4")
                nc.tensor.matmul(p4[:], wsb["WB1t"][:], S3[:], start=True, stop=True)
                B2 = wp.tile([48, TILE_N], mmdt, tag="B2")
                nc.vector.scalar_tensor_tensor(
                    B2[:], A2[0:48, :], 0.0, p4[:],
                    mybir.AluOpType.is_gt, mybir.AluOpType.mult)

                p5 = ps.tile([48, TILE_N], f32, tag="p5")
                nc.tensor.matmul(p5[:], wsb["WB2t"][:], B2[:], start=True, stop=True)
                B3 = wp.tile([48, TILE_N], mmdt, tag="B3")
                nc.vector.scalar_tensor_tensor(
                    B3[:], A0[0:48, :], 0.0, p5[:],
                    mybir.AluOpType.is_gt, mybir.AluOpType.mult)

                p6 = ps.tile([8, TILE_N], f32, tag="p6")
                nc.tensor.matmul(p6[:], wsb["P1t"][:], A3[:], start=True, stop=False)
                nc.tensor.matmul(p6[:], wsb["P2t"][:], B3[:], start=False, stop=False)
                nc.tensor.matmul(p6[:], wsb["P3t"][:], tinr[:], start=False, stop=True)
                OUT = wp.tile([8, TILE_N], f32, tag="OUT")
                nc.scalar.activation(OUT[:], p6[:],
                                     mybir.ActivationFunctionType.Copy)
                nc.sync.dma_start(dy[:, sl], OUT[:])

    nc.finalize()
    orig = nc.to_json_bytes
    nc.to_json_bytes = lambda: _split_waits(orig())
    return nc


_NC_CACHE = None


def kernel(nodes, edges, control, params):
    global _NC_CACHE
    nodes = np.asarray(nodes, dtype=np.float32)
    edges = np.asarray(edges, dtype=np.float32)
    control = np.asarray(control, dtype=np.float32)
    params = {k: [(np.asarray(w), np.asarray(b)) for (w, b) in v]
              for k, v in params.items()}

    mats = build_mats(params)
    IN_ALL = make_in_all(nodes, edges, control)  # [8, B]

    if _NC_CACHE is None:
        _NC_CACHE = build_nc()
    nc = _NC_CACHE

    in_maps = []
    for c in range(N_CORES):
        m = {k: np.ascontiguousarray(v) for k, v in mats.items()}
        m["inall"] = np.ascontiguousarray(IN_ALL[:, c * S:(c + 1) * S])
        in_maps.append(m)

    res = run_bass_kernel_spmd(nc, in_maps, core_ids=list(range(N_CORES)),
                               trace=bool(os.environ.get("LC_TRACE")))
    global _LAST_RESULTS
    _LAST_RESULTS = res
    out = np.concatenate([r["y8"].T for r in res.results], axis=0)
    return out.astype(np.float32)


_LAST_RESULTS = None
